# revision 41
# baseline (speedup 1.0000x reference)
"""Trainium2 Bass kernel for CrossViewDeformableBlock (sparse deformable attention).

Contract: kernel(**inputs) -> np.ndarray takes FULL inputs (as from
setup_inputs()) and returns the FULL output [b, 128, 64, 64].

Sharding: 8 cores, q-parallel. Core c handles batch b_c = c//4 and query
range [(c%4)*1024, +1024) of the 64*64=4096 BEV queries. Each core builds
bf16 K|V image tables for its 6 cameras on-device, computes projection /
offsets / bilinear sample coordinates on-device, gathers paired 2-position
rows with dma_gather, bilinear-blends corners (split across the DVE and
Act engines), does the point-softmax attention and output projection, and
writes its [1024, 128] output shard. The host only slices inputs,
transposes weights, and casts the image features / kv weights to bf16
(layout/dtype staging).

Algebraic simplifications vs the reference:
  - the K bias (bkv[:128]) shifts sim by a constant per (q, head) and
    cancels in the softmax -> dropped entirely;
  - the V bias (bkv[128:]) is a constant output offset (attention weights
    sum to 1) -> folded into the output-projection bias via one matmul;
  - pinhole projection composed as a single 3x4 matrix MT = (K E[:3])^T
    on PE, applied per q-tile with one matmul for all 6 cams.

Emission order (chosen so engine queues never head-of-line block):
  1. offsets + projection matmuls, batched per-cam DVE coordinate math;
  2. kv tables for cams 0/1: one whole-image DMA, 22 matmuls into an SBUF
     stage, 3 large pattern DMAs out (paired-row interleave in the dram
     access pattern); tables 2-5 are built the same way, spread across
     the attention iterations two cameras ahead of their use;
  3. gather index tables: partition rewrap via 8 SBUF->SBUF DMAs plus a
     replication matmul reading a permuted access pattern;
  4. attention loop, cam-outer / q-tile-inner: dma_gather of 2KB paired
     corner rows -> x/y lerp blends (LERP2 custom DVE op for some points,
     Act-scale pairs + DVE add for others) -> K.q dot + point softmax
     (exp on Act, 1/NCAM folded into the Act-side att expansion that also
     makes the V-weight multiply a fast-mode stride-1 op) -> V sum;
  5. per-q-tile output projection emitted as soon as its last cam is done.
"""

import math
import os
import numpy as np

import concourse.bass as bass
import concourse.mybir as mybir
import concourse.tile as tile
from concourse import bacc
from concourse.bass import ts
from concourse.masks import make_identity

# ---------------------------------------------------------------- constants
B, NCAM, H, W = 2, 6, 64, 64
HW = H * W                      # 4096 queries per batch
IH, IW = 32, 88                 # image feature h, w
IHW = IH * IW                   # 2816 positions
HEADS, DH, INNER = 4, 32, 128
NP = 8                          # sample points per query
DIM = 128
NCORES = 8
QPC = HW // (NCORES // B)       # 1024 queries per core
NQT = QPC // 128                # 8 q-tiles of 128
PADROWS = 2944                  # 23 * 128 rows in kv table (2816 + 128 pad)
KVROW = 2 * INNER               # 256 channels (K|V) per position
F32 = mybir.dt.float32
BF16 = mybir.dt.bfloat16
I16 = mybir.dt.int16


def _build_lerp2_2x_uop():
    """Hand-written 2x_1p uop program for LERP2 (T1 per the custom-DVE design
    doc): lo = Src0*C0 + Src1*C1 on blocks 0-2, hi = Src0_HI*C0 + Src1_HI*C1
    on blocks 3-5, lo carried to the output on delay chain 0."""
    from concourse.dve_uop import (
        UopConfig, InpSel, OutSel, OutPath, AluOp, AluInp, DelayInp, Trigger,
        ENABLE,
    )
    u = UopConfig()
    u.enable_input(InpSel.SRC_0, 1)      # chain 0
    u.enable_input(InpSel.CONST_0, 2)    # chain 1
    u.enable_input(InpSel.SRC_1, 3)      # chain 2
    u.enable_input(InpSel.CONST_1, 4)    # chain 3
    u.enable_input(InpSel.SRC_0_HI, 5)   # chain 4
    u.enable_input(InpSel.SRC_1_HI, 6)   # chain 5
    u.require_inp0 = ENABLE
    u.require_inp1 = ENABLE
    u.trigger = (Trigger.SRC_TENSOR_DONE, Trigger.NONE, Trigger.NONE)
    b = u.datapath_config
    b[0].enable_alu(AluOp.MULTIPLY, AluInp.PREV_DELAY_0, AluInp.PREV_DELAY_1)
    b[0].pass_through_delay(1, 2, 3, 4, 5)
    b[1].enable_alu(AluOp.MULTIPLY, AluInp.PREV_DELAY_2, AluInp.PREV_DELAY_3)
    b[1].enable_delay_from_src(DelayInp.PREV_ALU_OUT, 0)
    b[1].pass_through_delay(1, 3, 4, 5)
    b[2].enable_alu(AluOp.ADD, AluInp.PREV_DELAY_0, AluInp.PREV_ALU_OUT)
    b[2].pass_through_delay(1, 3, 4, 5)
    b[3].enable_alu(AluOp.MULTIPLY, AluInp.PREV_DELAY_4, AluInp.PREV_DELAY_1)
    b[3].enable_delay_from_src(DelayInp.PREV_ALU_OUT, 0)
    b[3].pass_through_delay(3, 5)
    b[4].enable_alu(AluOp.MULTIPLY, AluInp.PREV_DELAY_5, AluInp.PREV_DELAY_3)
    b[4].enable_delay_from_src(DelayInp.PREV_ALU_OUT, 1)
    b[4].pass_through_delay(0)
    b[5].enable_alu(AluOp.ADD, AluInp.PREV_DELAY_1, AluInp.PREV_ALU_OUT)
    b[5].pass_through_delay(0)
    b[6].pass_through_alu()
    b[6].pass_through_delay(0)
    b[7].pass_through_alu()
    b[7].pass_through_delay(0)
    u.enable_output(OutSel.DELAY_0, OutPath.WR0_LO)
    u.enable_output(OutSel.ALU_OUT, OutPath.WR0_HI)
    return u


def _register_lerp_op():
    """Register LERP2 (out = in0*s0 + in1*s1, per-partition scalar columns)
    with both the stock 1x program and the hand-written 2x program."""
    from concourse.dve_spec import Spec, Src0, Src1, C0, C1, lower
    from concourse.dve_spec import _has_src1 as has_src1
    from concourse.dve_uop import DveOpSpec
    from concourse.dve_ops import (DveOp, OPS, _SUB_OPCODE_FOR_NAME,
                                   _CUSTOM_DVE_ROW_BASE, _COMPILE_CACHE,
                                   get_dve_sub_opcode)
    from concourse import dve_ops as _do

    spec = Spec(
        body=Src0 * C0 + Src1 * C1,
        reference=lambda in0, in1, s0, s1, imm2: (
            in0.astype(np.float32) * s0 + in1.astype(np.float32) * s1
        ),
    )
    if "LERP2" not in _SUB_OPCODE_FOR_NAME:
        opcode = _CUSTOM_DVE_ROW_BASE + len(OPS)
        assert opcode < 0x20
        OPS.append(DveOp("LERP2", spec, subdim=False, uops_sha={}, perf_en={}))
        _SUB_OPCODE_FOR_NAME["LERP2"] = opcode
        _do.CUSTOM_DVE_SPECS["LERP2"] = spec
    op = next(o for o in OPS if o.name == "LERP2")
    key = ("LERP2", "v3")  # TRN2 = v3
    if key not in _COMPILE_CACHE:
        compiled = DveOpSpec(
            name="LERP2",
            opcode=get_dve_sub_opcode("LERP2"),
            uops=lower(spec, ver="v3"),
            uops_2x=[_build_lerp2_2x_uop()],
            perf_max=1,
            rd1_en=has_src1(spec),
        )
        compiled.validate("v3")
        _COMPILE_CACHE[key] = compiled
    return op


def build_kernel(nc):
    """Emit the SPMD program. All per-core variation comes via input data."""
    lerp_op = _register_lerp_op()
    dotp_op = _register_dotp_op()

    # ---------------- dram I/O ----------------
    img = nc.dram_tensor("img", [NCAM, DIM, IHW], BF16, kind="ExternalInput").ap()
    wkvT = nc.dram_tensor("wkvT", [DIM, KVROW], BF16, kind="ExternalInput").ap()
    bv_c = nc.dram_tensor("bv_c", [INNER, 1], F32, kind="ExternalInput").ap()
    bev_l = nc.dram_tensor("bev_l", [DIM, QPC], BF16, kind="ExternalInput").ap()
    wxy_l = nc.dram_tensor("wxy_l", [2, QPC], F32, kind="ExternalInput").ap()
    E_l = nc.dram_tensor("E_l", [4, NCAM * 4], F32, kind="ExternalInput").ap()
    KT = nc.dram_tensor("KT", [3, NCAM * 3], F32, kind="ExternalInput").ap()
    wqT = nc.dram_tensor("wqT", [DIM, INNER], BF16, kind="ExternalInput").ap()
    bq_r = nc.dram_tensor("bq_r", [1, INNER], BF16, kind="ExternalInput").ap()
    w1T = nc.dram_tensor("w1T", [DIM, DIM], BF16, kind="ExternalInput").ap()
    b1 = nc.dram_tensor("b1", [DIM, 1], F32, kind="ExternalInput").ap()
    w2T = nc.dram_tensor("w2T", [DIM, 2 * NP], BF16, kind="ExternalInput").ap()
    b2 = nc.dram_tensor("b2", [1, 2 * NP], BF16, kind="ExternalInput").ap()
    wpT = nc.dram_tensor("wpT", [INNER, DIM], F32, kind="ExternalInput").ap()
    bp_r = nc.dram_tensor("bp_r", [1, DIM], BF16, kind="ExternalInput").ap()
    cst01 = nc.dram_tensor("cst01", [2, QPC], F32, kind="ExternalInput").ap()
    rep_in = nc.dram_tensor("rep_in", [16, 128], F32, kind="ExternalInput").ap()
    out_l = nc.dram_tensor("out_l", [QPC, DIM], F32, kind="ExternalOutput").ap()

    with tile.TileContext(nc) as tc:
        _emit(tc, nc, lerp_op, dotp_op, img, wkvT, bv_c, bev_l, wxy_l, E_l,
              KT, wqT, bq_r, w1T, b1, w2T, b2, wpT, bp_r, cst01, rep_in, out_l)
    return nc


def _lerp(nc, lerp_op, out, in0, in1, s0, s1):
    """out = in0*s0 + in1*s1 with s0/s1 [P,1] columns (2x perf mode)."""
    r = nc.vector._custom_dve(lerp_op, out=out, in0=in0, in1=in1, s0=s0, s1=s1)
    r.ins.perf_max = 1
    return r


def _build_dotp_uop(kind, two_x):
    """One DOTP uop: grouped dot product via segment-reset inclusive scan.
    kind: 'init' (acc=0, no consume) | 'steady' (acc+=prod) | 'reseed'
    (acc=prod at each subdim boundary). The out stream carries the running
    prefix every element; the caller's out AP picks where prefixes land."""
    from concourse.dve_uop import (
        UopConfig, InpSel, OutSel, OutPath, AluOp, AluInp, DelayInp,
        Trigger, ENABLE,
    )
    u = UopConfig()
    if two_x:
        u.enable_input(InpSel.SRC_0, 1)
        u.enable_input(InpSel.SRC_1, 2)
        u.enable_input(InpSel.SRC_0_HI, 3)
        u.enable_input(InpSel.SRC_1_HI, 4)
        u.enable_input(InpSel.ZERO, 5)
        zchain = AluInp.PREV_DELAY_4
        acc_blk = 3
    else:
        u.enable_input(InpSel.SRC_0, 1)
        u.enable_input(InpSel.SRC_1, 2)
        u.enable_input(InpSel.ZERO, 3)
        zchain = AluInp.PREV_DELAY_2
        acc_blk = 1
    if kind == "init":
        u.repeat_count = 1
        u.trigger = (Trigger.COUNT, Trigger.NONE, Trigger.NONE)
        u.next_uop = (1, 0, 0)
    else:
        u.require_inp0 = ENABLE
        u.require_inp1 = ENABLE
        u.enable_output(OutSel.ALU_OUT, OutPath.WR0_LO)
        if two_x:
            u.enable_output(OutSel.ALU_OUT, OutPath.WR0_HI)
        if kind == "steady":
            u.trigger = (Trigger.SRC_TENSOR_DONE, Trigger.SUB_DIM_DONE,
                         Trigger.NONE)
            u.next_uop = (0, 2, 0)
        else:  # reseed
            u.repeat_count = 1
            u.trigger = (Trigger.SRC_TENSOR_DONE, Trigger.SUB_DIM_DONE,
                         Trigger.COUNT)
            u.next_uop = (0, 2, 1)
    b = u.datapath_config
    b[0].enable_alu(AluOp.MULTIPLY, AluInp.PREV_DELAY_0, AluInp.PREV_DELAY_1)
    if two_x:
        b[0].pass_through_delay(2, 3, 4)
        b[1].enable_alu(AluOp.MULTIPLY, AluInp.PREV_DELAY_2, AluInp.PREV_DELAY_3)
        b[1].enable_delay_from_src(DelayInp.PREV_ALU_OUT, 0)
        b[1].pass_through_delay(4)
        b[2].enable_alu(AluOp.ADD, AluInp.PREV_DELAY_0, AluInp.PREV_ALU_OUT)
        b[2].pass_through_delay(4)
    else:
        b[0].pass_through_delay(2)
    if kind == "init":
        b[acc_blk].enable_alu(AluOp.BYPASS, zchain, zchain)
    elif kind == "steady":
        b[acc_blk].enable_alu(AluOp.ADD, AluInp.CURR_ALU_OUT,
                              AluInp.PREV_ALU_OUT)
    else:
        b[acc_blk].enable_alu(AluOp.BYPASS, AluInp.PREV_ALU_OUT,
                              AluInp.PREV_ALU_OUT)
    for i in range(acc_blk + 1, 8):
        b[i].pass_through_alu()
    return u


def _register_dotp_op():
    """Register DOTP (grouped dot product, 1x + 2x programs)."""
    from concourse.dve_spec import Spec, Src0, Src1, scan
    from concourse.dve_uop import DveOpSpec, AluOp
    from concourse.dve_ops import (DveOp, OPS, _SUB_OPCODE_FOR_NAME,
                                   _CUSTOM_DVE_ROW_BASE, _COMPILE_CACHE,
                                   get_dve_sub_opcode)
    from concourse import dve_ops as _do

    spec = Spec(
        body=scan(AluOp.ADD, Src0 * Src1),
        reference=lambda in0, in1, s0, s1, imm2: np.cumsum(
            in0.astype(np.float32) * in1.astype(np.float32), axis=-1
        ),
    )
    if "DOTP" not in _SUB_OPCODE_FOR_NAME:
        opcode = _CUSTOM_DVE_ROW_BASE + len(OPS)
        assert opcode < 0x20
        OPS.append(DveOp("DOTP", spec, subdim=True, uops_sha={}, perf_en={}))
        _SUB_OPCODE_FOR_NAME["DOTP"] = opcode
        _do.CUSTOM_DVE_SPECS["DOTP"] = spec
    op = next(o for o in OPS if o.name == "DOTP")
    key = ("DOTP", "v3")
    if key not in _COMPILE_CACHE:
        compiled = DveOpSpec(
            name="DOTP",
            opcode=get_dve_sub_opcode("DOTP"),
            uops=[_build_dotp_uop("init", False),
                  _build_dotp_uop("steady", False),
                  _build_dotp_uop("reseed", False)],
            uops_2x=[_build_dotp_uop("init", True),
                     _build_dotp_uop("steady", True),
                     _build_dotp_uop("reseed", True)],
            perf_max=1,
            rd1_en=True,
        )
        compiled.validate("v3")
        _COMPILE_CACHE[key] = compiled
    return op


def _dotp(nc, op, out, in0, in1):
    r = nc.vector._custom_dve(op, out=out, in0=in0, in1=in1)
    r.ins.perf_max = 1
    return r


def _emit(tc, nc, lerp_op, dotp_op, img, wkvT, bv_c, bev_l, wxy_l, E_l, KT,
          wqT, bq_r, w1T, b1, w2T, b2, wpT, bp_r, cst01, rep_in, out_l):
    import contextlib
    ctx = contextlib.ExitStack()
    with ctx:
        singles = ctx.enter_context(tc.tile_pool(name="singles", bufs=1))
        cpool = ctx.enter_context(tc.tile_pool(name="cpool", bufs=1))
        temps = ctx.enter_context(tc.tile_pool(name="temps", bufs=2))
        stpool = ctx.enter_context(tc.tile_pool(name="stpool", bufs=1))
        gath = ctx.enter_context(tc.tile_pool(name="gath", bufs=4))
        blend = ctx.enter_context(tc.tile_pool(name="blend", bufs=2))
        stats = ctx.enter_context(tc.tile_pool(name="stats", bufs=4))
        psum = ctx.enter_context(tc.tile_pool(name="psum", bufs=3, space="PSUM"))
        psum2 = ctx.enter_context(tc.tile_pool(name="psum2", bufs=2, space="PSUM"))
        dram = ctx.enter_context(tc.tile_pool(name="dram", bufs=1, space="DRAM"))

        AX = mybir.AxisListType
        ALU = mybir.AluOpType
        ACTF = mybir.ActivationFunctionType

        # ------------- resident tiles -------------
        ident = singles.tile([128, 128], F32)
        make_identity(nc, ident[:])
        wkvT_sb = singles.tile([DIM, KVROW], BF16)
        nc.sync.dma_start(out=wkvT_sb[:], in_=wkvT)
        bvc_sb = singles.tile([INNER, 1], F32)
        nc.sync.dma_start(out=bvc_sb[:], in_=bv_c)
        bev_sb = singles.tile([DIM, QPC], BF16)
        nc.sync.dma_start(out=bev_sb[:], in_=bev_l)
        wqT_sb = singles.tile([DIM, INNER], BF16)
        nc.sync.dma_start(out=wqT_sb[:], in_=wqT)
        bq_sb = singles.tile([1, INNER], BF16)
        nc.sync.dma_start(out=bq_sb[:], in_=bq_r)
        w1T_sb = singles.tile([DIM, DIM], BF16)
        nc.sync.dma_start(out=w1T_sb[:], in_=w1T)
        w2T_sb = singles.tile([DIM, 2 * NP], BF16)
        nc.sync.dma_start(out=w2T_sb[:], in_=w2T)
        wpT_sb = singles.tile([INNER, DIM], F32)
        nc.sync.dma_start(out=wpT_sb[:], in_=wpT)
        bp_sb = singles.tile([1, DIM], BF16)
        nc.sync.dma_start(out=bp_sb[:], in_=bp_r)
        b1_sb = singles.tile([DIM, 1], F32)
        nc.sync.dma_start(out=b1_sb[:], in_=b1)
        b2r_sb = singles.tile([1, 2 * NP], BF16)
        nc.sync.dma_start(out=b2r_sb[:], in_=b2)
        E_sb = singles.tile([4, NCAM * 4], F32)
        nc.sync.dma_start(out=E_sb[:], in_=E_l)
        KT_sb = singles.tile([3, NCAM * 3], F32)
        nc.sync.dma_start(out=KT_sb[:], in_=KT)
        REP_sb = singles.tile([16, 128], F32)
        nc.sync.dma_start(out=REP_sb[:], in_=rep_in)
        ones_bf = singles.tile([1, 128], BF16)
        nc.vector.memset(ones_bf[:], 1.0)

        # xyz1 = [wx, wy, 0, 1]
        xyz1_sb = singles.tile([4, QPC], F32)
        nc.sync.dma_start(out=xyz1_sb[:2, :], in_=wxy_l)
        nc.sync.dma_start(out=xyz1_sb[2:4, :], in_=cst01)

        # resident products of phase A
        qbf_sb = singles.tile([128, QPC], BF16)
        off_t_all = singles.tile([128, NQT * 16], F32)
        wA_sb = singles.tile([128, NCAM * NQT * 16], F32)
        wB_sb = singles.tile([128, NCAM * NQT * 16], F32)
        idx2_all = singles.tile([128, NCAM * NQT * NP], F32)
        wacc_all = singles.tile([128, NQT * INNER], F32)
        T_tiles = [singles.tile([128, NCAM * 64], I16, tag=f"Tq{qt}",
                                name=f"Tq{qt}")
                   for qt in range(NQT)]

        # per-cam kv tables in DRAM; row y*IW+x holds KV(y,x) ++ KV(y+1,x)
        kv_cam = [dram.tile([PADROWS, 2 * KVROW], BF16, tag=f"kv{c}",
                            name=f"kv{c}")
                  for c in range(NCAM)]

        # ---------------- kv table builder ----------------
        zt = singles.tile([128, 2 * KVROW], BF16)
        nc.vector.memset(zt[:], 0)
        NPT = IHW // 128  # 22 position tiles

        # zero the tail pad rows (IHW..PADROWS) of every cam upfront
        for _c in range(NCAM):
            kd = kv_cam[_c]
            nc.sync.dma_start(out=kd[IHW:PADROWS, :], in_=zt[:])

        # Table build, partition-blocked: partition p of the SBUF stage holds
        # table rows p*NPT..p*NPT+NPT-1 (col t = row p*NPT+t), so the store
        # is one DMA of 128 fully contiguous ~22KB runs. The paired second
        # half (row r ++ row r+IW) comes from a second matmul over an
        # IW-shifted column slice of the image. Store goes through the
        # gpsimd software DGE so it spreads across all 16 DMA rings instead
        # of serializing the sync/act sequencer queues.
        build_state = {}
        ROWE = 2 * KVROW  # 512 elements per table row

        def build_start(cam):
            # img padded with a zero tail so the +IW shifted slice stays
            # in-bounds (positions IHW..IHW+128 read as zero -> KV pad = 0)
            img_sb = stpool.tile([128, IHW + 128], BF16, tag="imgsb",
                                 name="imgsb")
            nc.vector.memset(img_sb[:, IHW:], 0)
            nc.scalar.dma_start(out=img_sb[:, 0:IHW], in_=img[cam])
            stage = stpool.tile([128, NPT, ROWE], BF16, tag="stage",
                                name="stage")
            build_state[cam] = (img_sb, stage)

        def build_chunk(cam, lo, hi, split_copies=False):
            img_sb, stage = build_state[cam]
            iap = img_sb[:]
            for t in range(lo, min(hi, NPT)):
                kv_ps = psum.tile([128, ROWE], F32, tag="mm2")
                # positions p*NPT+t (stride-NPT column slice), and +IW pair
                for j, off in enumerate((t, t + IW)):
                    lhsT = bass.AP(tensor=img_sb.tensor,
                                   offset=iap.offset + off,
                                   ap=[iap.ap[0], [NPT, 128]])
                    nc.tensor.matmul(out=kv_ps[:, ts(j, KVROW)], lhsT=lhsT,
                                     rhs=wkvT_sb[:], start=True, stop=True)
                if split_copies and t % 2 == 0:
                    nc.vector.tensor_copy(out=stage[:, t, :], in_=kv_ps[:])
                else:
                    nc.scalar.activation(out=stage[:, t, :], in_=kv_ps[:],
                                         func=ACTF.Copy)

        def build_store(cam):
            img_sb, stage = build_state.pop(cam)
            kd = kv_cam[cam]
            sap = stage[:]
            # dram row p*NPT+t at offset (p*NPT+t)*ROWE: per partition one
            # contiguous NPT*ROWE run
            out1 = bass.AP(tensor=kd.tensor, offset=kd[:].offset,
                           ap=[[NPT * ROWE, 128], [1, NPT * ROWE]])
            in1 = bass.AP(tensor=stage.tensor, offset=sap.offset,
                          ap=[sap.ap[0], [1, NPT * ROWE]])
            nc.gpsimd.dma_start(out=out1, in_=in1)

        # cam 0/1 tables first: their matmuls/copies/DMAs have no deps on
        # phase A, so PE/Act/DMA stream them at full rate while nothing else
        # is ready; phase A's engine ping-pong then overlaps the store DMAs
        # offsets: o1 = relu(w1 @ bev + b1); off = w2 @ o1 + b2  [16, QPC]
        o1_sb = singles.tile([DIM, QPC], BF16)
        for hf in range(2):
            o1_ps = psum2.tile([DIM, QPC // 2], F32, tag="wide")
            nc.tensor.matmul(out=o1_ps[:], lhsT=w1T_sb[:],
                             rhs=bev_sb[:, ts(hf, QPC // 2)], start=True, stop=True)
            nc.scalar.activation(out=o1_sb[:, ts(hf, QPC // 2)], in_=o1_ps[:],
                                 func=ACTF.Relu, bias=b1_sb[:], scale=1.0)
        # transposed offsets directly: off_t[q, o] = sum_d o1[d,q] w2T[d,o]
        # + b2 (bias accumulated via the ones-row matmul trick)
        for qt in range(NQT):
            ot_ps = psum.tile([128, 2 * NP], F32, tag="mm")
            nc.tensor.matmul(out=ot_ps[:], lhsT=ones_bf[:], rhs=b2r_sb[:],
                             start=True, stop=False)
            nc.tensor.matmul(out=ot_ps[:], lhsT=o1_sb[:, ts(qt, 128)],
                             rhs=w2T_sb[:], start=False, stop=True)
            nc.vector.tensor_copy(out=off_t_all[:, ts(qt, 2 * NP)],
                                  in_=ot_ps[:])

        # ---------------- A: per-cam projection + coords ----------------
        BIGF = 8388608.0
        halfx = 0.5 * (IW - 1)
        halfy = 0.5 * (IH - 1)
        # MT = (K @ E[:3,:])^T [4,3] per cam, computed directly:
        # MT[j,i] = sum_k E[k,j] K[i,k] = matmul(lhsT=E_rows, rhs=K^T)
        MT_sb = singles.tile([4, NCAM * 3], F32)
        for cam in range(NCAM):
            mt_ps = psum.tile([4, 3], F32, tag="mm")
            nc.tensor.matmul(out=mt_ps[:], lhsT=E_sb[0:3, ts(cam, 4)],
                             rhs=KT_sb[:, ts(cam, 3)], start=True, stop=True)
            nc.scalar.activation(out=MT_sb[:, ts(cam, 3)], in_=mt_ps[:],
                                 func=ACTF.Copy)

        # pxt_all [128, (qt, cam, 3)]: all cams' projections in one matmul/qt
        pxt_all = singles.tile([128, NQT * NCAM * 3], F32)
        for qt in range(NQT):
            pt_ps = psum.tile([128, NCAM * 3], F32, tag="mm")
            nc.tensor.matmul(out=pt_ps[:], lhsT=xyz1_sb[:, ts(qt, 128)],
                             rhs=MT_sb[:], start=True, stop=True)
            nc.vector.tensor_copy(out=pxt_all[:, ts(qt, NCAM * 3)],
                                  in_=pt_ps[:])

        pap = pxt_all[:]
        oap = off_t_all[:]

        def emit_coords():
            # all NCAM cameras in one batched DVE chain (minimizes the
            # dependent-op latency on the startup critical path)
            NC6 = NCAM

            def _px(col):  # [128, (cam, qt)] slice of pxt_all (qt, cam, 3)
                return bass.AP(tensor=pxt_all.tensor,
                               offset=pap.offset + col,
                               ap=[pap.ap[0], [3, NC6], [NCAM * 3, NQT]])

            # rden = 1 / max(pz, 1e-6)
            rden = cpool.tile([128, NC6, NQT], F32, tag="rden")
            nc.vector.tensor_scalar(out=rden[:], in0=_px(2), scalar1=1e-6,
                                    scalar2=None, op0=ALU.max)
            nc.vector.reciprocal(out=rden[:], in_=rden[:])
            # g = uv/(dim-1)*2 - 1
            gx = cpool.tile([128, NC6, NQT], F32, tag="gx")
            nc.vector.tensor_tensor(out=gx[:], in0=_px(0), in1=rden[:],
                                    op=ALU.mult)
            nc.vector.tensor_scalar(out=gx[:], in0=gx[:],
                                    scalar1=2.0 / (IW - 1), scalar2=1.0,
                                    op0=ALU.mult, op1=ALU.subtract)
            gy = cpool.tile([128, NC6, NQT], F32, tag="gy")
            nc.vector.tensor_tensor(out=gy[:], in0=_px(1), in1=rden[:],
                                    op=ALU.mult)
            nc.vector.tensor_scalar(out=gy[:], in0=gy[:],
                                    scalar1=2.0 / (IH - 1), scalar2=1.0,
                                    op0=ALU.mult, op1=ALU.subtract)

            # sxy [128, (cam, qt, 16)]: samp = clip(off+g, -1, 1) -> pixels
            sxy = cpool.tile([128, NC6 * NQT * 16], F32, tag="sxy")
            sap = sxy[:]

            def _sl(t, tap, off0):  # [128, (cam, qt, 8)] x(0)/y(8) slices
                return bass.AP(tensor=t.tensor, offset=tap.offset + off0,
                               ap=[tap.ap[0], [NQT * 16, NC6], [16, NQT],
                                   [1, NP]])

            def _obc(off0):  # off_t_all bc over the cams
                return bass.AP(tensor=off_t_all.tensor,
                               offset=oap.offset + off0,
                               ap=[oap.ap[0], [0, NC6], [16, NQT], [1, NP]])

            def _gbc(g):  # gx/gy bc over p
                gp = g[:]
                return bass.AP(tensor=g.tensor, offset=gp.offset,
                               ap=[gp.ap[0], [NQT, NC6], [1, NQT], [0, NP]])

            nc.vector.tensor_tensor(out=_sl(sxy, sap, 0), in0=_obc(0),
                                    in1=_gbc(gx), op=ALU.add)
            nc.vector.tensor_tensor(out=_sl(sxy, sap, NP), in0=_obc(NP),
                                    in1=_gbc(gy), op=ALU.add)
            nc.vector.tensor_scalar(out=sxy[:], in0=sxy[:], scalar1=1.0,
                                    scalar2=-1.0, op0=ALU.min, op1=ALU.max)
            nc.vector.tensor_scalar(out=_sl(sxy, sap, 0), in0=_sl(sxy, sap, 0),
                                    scalar1=1.0, scalar2=halfx,
                                    op0=ALU.add, op1=ALU.mult)
            nc.vector.tensor_scalar(out=_sl(sxy, sap, NP), in0=_sl(sxy, sap, NP),
                                    scalar1=1.0, scalar2=halfy,
                                    op0=ALU.add, op1=ALU.mult)

            # floor via +2^23 round-to-nearest, then fixup so frac >= 0
            rnd = cpool.tile([128, NC6 * NQT * 16], F32, tag="rnd")
            nc.vector.tensor_scalar(out=rnd[:], in0=sxy[:], scalar1=BIGF,
                                    scalar2=BIGF, op0=ALU.add, op1=ALU.subtract)
            dfr = cpool.tile([128, NC6 * NQT * 16], F32, tag="dfr")
            nc.vector.tensor_tensor(out=dfr[:], in0=sxy[:], in1=rnd[:],
                                    op=ALU.subtract)
            msk = cpool.tile([128, NC6 * NQT * 16], F32, tag="msk")
            nc.vector.tensor_scalar(out=msk[:], in0=dfr[:], scalar1=0.0,
                                    scalar2=None, op0=ALU.is_lt)
            x0y0 = sxy  # sxy is dead after dfr; reuse its buffer
            nc.vector.tensor_tensor(out=x0y0[:], in0=rnd[:], in1=msk[:],
                                    op=ALU.subtract)
            nc.vector.tensor_tensor(out=wB_sb[:, 0:NC6 * NQT * 16],
                                    in0=dfr[:], in1=msk[:], op=ALU.add)
            nc.vector.tensor_scalar(out=wA_sb[:, 0:NC6 * NQT * 16],
                                    in0=wB_sb[:, 0:NC6 * NQT * 16],
                                    scalar1=-1.0, scalar2=1.0,
                                    op0=ALU.mult, op1=ALU.add)
            # idx = y0*IW + x0 (local per cam); idx2_all layout (qt, cam, p)
            xap = x0y0[:]
            rap = rnd[:]  # rnd is dead after x0y0; reuse its buffer for tmp
            tmp = bass.AP(tensor=rnd.tensor, offset=rap.offset,
                          ap=[rap.ap[0], [NQT * NP, NC6], [NP, NQT], [1, NP]])
            nc.vector.tensor_scalar(out=tmp, in0=_sl(x0y0, xap, NP),
                                    scalar1=float(IW), scalar2=None,
                                    op0=ALU.mult)
            i2 = idx2_all[:]
            idst = bass.AP(tensor=idx2_all.tensor, offset=i2.offset,
                           ap=[i2.ap[0], [NP, NC6], [NCAM * NP, NQT], [1, NP]])
            nc.vector.tensor_tensor(out=idst, in0=tmp,
                                    in1=_sl(x0y0, xap, 0), op=ALU.add)

        # ---------------- A: queries ----------------
        for qt in range(NQT):
            q_ps = psum.tile([128, INNER], F32, tag="mm")
            nc.tensor.matmul(out=q_ps[:], lhsT=ones_bf[:], rhs=bq_sb[:],
                             start=True, stop=False)
            nc.tensor.matmul(out=q_ps[:], lhsT=bev_sb[:, ts(qt, 128)],
                             rhs=wqT_sb[:], start=False, stop=True)
            nc.scalar.activation(out=qbf_sb[:, ts(qt, INNER)], in_=q_ps[:],
                                 func=ACTF.Copy)

        # fold the V bias through the output projection: bpp = bp + bv @ wpT
        # (bk cancels in the softmax; bv is a constant output offset since the
        # attention weights sum to 1)
        bvp_ps = psum.tile([1, DIM], F32, tag="mm")
        nc.tensor.matmul(out=bvp_ps[:], lhsT=bvc_sb[:], rhs=wpT_sb[:],
                         start=True, stop=True)
        bpp_sb = singles.tile([1, DIM], BF16)
        nc.vector.tensor_tensor(out=bpp_sb[:], in0=bvp_ps[:], in1=bp_sb[:],
                                op=ALU.add)

        # ---------------- B: gather index tables (per cam-pair) ----------
        # Need T[16k+pl, cam*64 + p*8 + qh] = idx2_all[qh*16+pl, (qt, cam, p)].
        # Per pair: 8 SBUF->SBUF DMAs rewrap partitions into a [16,
        # (qh,qt,cam2,p)] scratch; a replication matmul reading a permuted
        # access pattern writes the pair's T_tiles columns. Pair 0 runs
        # before the loop; pairs 1/2 are emitted inside the first camera's
        # iterations (their coords overlap the first gathers).
        tsc = singles.tile([16, 8 * NQT * NCAM * NP], F32)

        def emit_rewrap():
            for qh in range(8):
                nc.sync.dma_start(
                    out=tsc[:, qh * 384:(qh + 1) * 384],
                    in_=idx2_all[qh * 16:(qh + 1) * 16, :])

        def emit_ttables(qts):
            tap = tsc[:]
            for qt in qts:
                rhs_perm = bass.AP(tensor=tsc.tensor,
                                   offset=tap.offset + qt * 48,
                                   ap=[tap.ap[0], [NP, NCAM], [1, NP],
                                       [384, 8]])
                rep_ps = psum2.tile([128, NCAM * 64], F32, tag="wide")
                nc.tensor.matmul(out=rep_ps[:], lhsT=REP_sb[:],
                                 rhs=rhs_perm, start=True, stop=True)
                nc.vector.tensor_copy(out=T_tiles[qt][:], in_=rep_ps[:])

        # startup pipeline: batched coords for all cams overlap the cam-0
        # table build (coords on DVE, build on PE/Act/V-copies)
        emit_coords()
        build_start(0)
        build_chunk(0, 0, NPT)
        build_store(0)
        emit_rewrap()
        emit_ttables(list(range(NQT)))
        build_start(1)
        build_chunk(1, 0, NPT)
        build_store(1)

        # ---------------- C/D/E: attention, cam-outer ----------------
        # cam c+1's kv table tiles are emitted spread across cam c's q-tile
        # iterations so their psum->sbuf copies never head-of-line-block the
        # Act engine ahead of the softmax exp.
        # All blends run as 2x-mode LERP2 on the DVE; the Act engine only
        # handles exp/atx/table copies.

        for cam in range(NCAM):
            camv = kv_cam[cam]
            kv_view = bass.AP(tensor=camv.tensor, offset=camv[:].offset,
                              ap=[[2 * KVROW, PADROWS - 1], [1, 4 * KVROW]])
            for qt in range(NQT):
                if cam + 2 < NCAM:
                    # table c+2 spread over cam c's iterations: it is complete
                    # one full camera before its gathers start, so the gather
                    # prefetch never stalls on a table write
                    if qt == 0:
                        build_start(cam + 2)
                    build_chunk(cam + 2, qt * 3, qt * 3 + 3)
                    if qt == NQT - 1:
                        build_store(cam + 2)
                kvraw = gath.tile([128, NP, 4 * KVROW], BF16, tag="kvraw")
                nc.gpsimd.dma_gather(
                    out_ap=kvraw[:], in_ap=kv_view,
                    idxs_ap=T_tiles[qt][:, ts(cam, 64)],
                    num_idxs=1024, num_idxs_reg=1024,
                    elem_size=4 * KVROW, elem_step=2 * KVROW,
                    single_packet=False)
                # x-blend: 8 rows of 512 [(y0,y1) x (K|V)]
                blkw = (cam * NQT + qt) * 16
                kvx = blend.tile([128, NP, 2 * KVROW], BF16, tag="kvx")
                for p in range(NP):
                    sa = wA_sb[:, blkw + p:blkw + p + 1]
                    sb = wB_sb[:, blkw + p:blkw + p + 1]
                    _lerp(nc, lerp_op, kvx[:, p, :],
                          kvraw[:, p, 0:2 * KVROW],
                          kvraw[:, p, 2 * KVROW:4 * KVROW], sa, sb)
                # y-blend: 8 points of 256, written as split K/V planes
                # (kvb2[:,0] = K [128, NP, DH*HEADS] contiguous, kvb2[:,1] = V)
                kvb2 = blend.tile([128, 2, NP, INNER], BF16, tag="kvb2")
                k2ap = kvb2[:]
                for p in range(NP):
                    sa = wA_sb[:, blkw + 8 + p:blkw + 9 + p]
                    sb = wB_sb[:, blkw + 8 + p:blkw + 9 + p]
                    yout = bass.AP(tensor=kvb2.tensor,
                                   offset=k2ap.offset + p * INNER,
                                   ap=[k2ap.ap[0], [NP * INNER, 2], [1, INNER]])
                    if p >= 4:
                        tA = blend.tile([128, KVROW], BF16, tag="ya")
                        nc.scalar.activation(out=tA[:], in_=kvx[:, p, 0:KVROW],
                                             func=ACTF.Copy, scale=sa)
                        tB = blend.tile([128, KVROW], BF16, tag="yb")
                        nc.scalar.activation(out=tB[:],
                                             in_=kvx[:, p, KVROW:2 * KVROW],
                                             func=ACTF.Copy, scale=sb)
                        nc.vector.tensor_tensor(out=yout, in0=tA[:],
                                                in1=tB[:], op=ALU.add)
                    else:
                        _lerp(nc, lerp_op, yout,
                              kvx[:, p, 0:KVROW], kvx[:, p, KVROW:2 * KVROW],
                              sa, sb)
                # sim via fused 2x dot-product scan: running prefix lands in
                # simsc; group sums (p,h) at column g*DH+DH-1
                simsc = blend.tile([128, NP * INNER], BF16, tag="simsc")
                qv = qbf_sb[:, ts(qt, INNER)]
                _dotp(nc, dotp_op, simsc[:],
                      bass.AP(tensor=kvb2.tensor, offset=k2ap.offset,
                              ap=[k2ap.ap[0], [DH, NP * HEADS], [1, DH]]),
                      bass.AP(tensor=qbf_sb.tensor, offset=qv.offset,
                              ap=[qv.ap[0], [0, NP], [1, INNER]]))
                # softmax over p; per-head normalization folded into the
                # Act-engine expansion scale (1/NCAM is folded into wkv's V
                # half on the host)
                scap = simsc[:]
                esim = stats.tile([128, NP, HEADS], BF16, tag="esim")
                nc.scalar.activation(
                    out=esim[:],
                    in_=bass.AP(tensor=simsc.tensor, offset=scap.offset + DH - 1,
                                ap=[scap.ap[0], [DH, NP * HEADS]]),
                    func=ACTF.Exp)
                ssum = stats.tile([128, HEADS], F32, tag="ssum")
                esap = esim[:]
                nc.vector.tensor_reduce(
                    out=ssum[:],
                    in_=bass.AP(tensor=esim.tensor, offset=esap.offset,
                                ap=[esap.ap[0], [1, HEADS], [HEADS, NP]]),
                    axis=AX.X, op=ALU.add)
                srec = stats.tile([128, HEADS], F32, tag="srec")
                nc.vector.reciprocal(out=srec[:], in_=ssum[:])
                # expand att = esim/ssum over DH: one scaled copy per head
                atx = blend.tile([128, NP, HEADS, DH], BF16, tag="atx")
                atxap = atx[:]
                for h in range(HEADS):
                    nc.scalar.activation(
                        out=bass.AP(tensor=atx.tensor,
                                    offset=atxap.offset + h * DH,
                                    ap=[atxap.ap[0], [HEADS * DH, NP], [1, DH]]),
                        in_=bass.AP(tensor=esim.tensor, offset=esap.offset + h,
                                    ap=[esap.ap[0], [HEADS, NP], [0, DH]]),
                        func=ACTF.Copy, scale=srec[:, h:h + 1])
                vw = blend.tile([128, NP, INNER], BF16, tag="vw")
                nc.vector.tensor_tensor(out=vw[:], in0=kvb2[:, 1, :, :],
                                        in1=atx[:], op=ALU.mult)
                # tree-sum over the 8 points
                t1 = stats.tile([128, 4, INNER], BF16, tag="t1")
                nc.vector.tensor_tensor(out=t1[:], in0=vw[:, 0:4, :],
                                        in1=vw[:, 4:8, :], op=ALU.add)
                t2 = stats.tile([128, 2, INNER], BF16, tag="t2")
                nc.vector.tensor_tensor(out=t2[:], in0=t1[:, 0:2, :],
                                        in1=t1[:, 2:4, :], op=ALU.add)
                if cam == 0:
                    nc.vector.tensor_tensor(out=wacc_all[:, ts(qt, INNER)],
                                            in0=t2[:, 0, :], in1=t2[:, 1, :],
                                            op=ALU.add)
                else:
                    wsum = stats.tile([128, INNER], BF16, tag="wsum")
                    nc.vector.tensor_tensor(out=wsum[:], in0=t2[:, 0, :],
                                            in1=t2[:, 1, :], op=ALU.add)
                    nc.vector.tensor_tensor(out=wacc_all[:, ts(qt, INNER)],
                                            in0=wacc_all[:, ts(qt, INNER)],
                                            in1=wsum[:], op=ALU.add)
                if cam == NCAM - 1:
                    # output projection for this q-tile, overlapped with the
                    # remaining iterations
                    wt_ps = psum.tile([128, 128], F32, tag="mm")
                    nc.tensor.transpose(out=wt_ps[:],
                                        in_=wacc_all[:, ts(qt, INNER)],
                                        identity=ident[:])
                    waccT = temps.tile([128, 128], F32, tag="waccT")
                    nc.scalar.activation(out=waccT[:], in_=wt_ps[:],
                                         func=ACTF.Copy)
                    out_ps = psum.tile([128, DIM], F32, tag="mm")
                    nc.tensor.matmul(out=out_ps[:], lhsT=ones_bf[:],
                                     rhs=bpp_sb[:], start=True, stop=False)
                    nc.tensor.matmul(out=out_ps[:], lhsT=waccT[:],
                                     rhs=wpT_sb[:], start=False, stop=True)
                    outf = temps.tile([128, DIM], F32, tag="outf")
                    nc.scalar.activation(out=outf[:], in_=out_ps[:],
                                         func=ACTF.Copy)
                    nc.sync.dma_start(out=out_l[ts(qt, 128), :], in_=outf[:])


# ---------------------------------------------------------------- host side
_CACHED = {}


def _build():
    if "nc" not in _CACHED:
        nc = bacc.Bacc("TRN2", target_bir_lowering=False, debug=False,
                       num_devices=NCORES)
        build_kernel(nc)
        nc.compile()
        _CACHED["nc"] = nc
    return _CACHED["nc"]


def make_in_maps(inputs):
    """Slice/transpose/cast FULL inputs into 8 per-core input dicts."""
    import ml_dtypes
    BF = ml_dtypes.bfloat16
    f = lambda x: np.ascontiguousarray(np.asarray(x, dtype=np.float32))
    bev = f(inputs["bev"]).reshape(B, DIM, HW)
    img_feats = f(inputs["img_feats"]).reshape(B, NCAM, DIM, IHW)
    Kc = f(inputs["K"])
    Ec = f(inputs["E"])
    world_xy = f(inputs["world_xy"]).reshape(2, HW)
    wq = f(inputs["wq"]); bq = f(inputs["bq"])
    wkv = f(inputs["wkv"]); bkv = f(inputs["bkv"])
    w_off1 = f(inputs["w_off1"]); b_off1 = f(inputs["b_off1"])
    w_off2 = f(inputs["w_off2"]); b_off2 = f(inputs["b_off2"])
    w_proj = f(inputs["w_proj"]); b_proj = f(inputs["b_proj"])

    # row-permute w_off2/b_off2 from (p, c) to (c, p) ordering
    perm = [p * 2 + c for c in range(2) for p in range(NP)]
    w2p = w_off2[perm, :]
    b2p = b_off2[perm]

    # fold the 1/NCAM camera mean into the V projection (attention weights
    # sum to 1 per cam, so only V and its bias carry the mean; the bias fold
    # bv @ wpT is unaffected since sum över cams restores the factor NCAM)
    wkv_f = wkv.copy()
    wkv_f[INNER:, :] *= 1.0 / NCAM

    in_maps = []
    for core in range(NCORES):
        bc = core // (NCORES // B)
        q0 = (core % (NCORES // B)) * QPC
        m = {
            "img": np.ascontiguousarray(img_feats[bc]).astype(BF),
            "wkvT": np.ascontiguousarray(wkv_f.T).astype(BF),
            "bv_c": bkv[INNER:].reshape(INNER, 1),
            "bev_l": np.ascontiguousarray(bev[bc, :, q0:q0 + QPC]).astype(BF),
            "wxy_l": np.ascontiguousarray(world_xy[:, q0:q0 + QPC]),
            "E_l": np.ascontiguousarray(Ec[bc].transpose(1, 0, 2).reshape(4, NCAM * 4)),
            "KT": np.ascontiguousarray(Kc[bc].transpose(2, 0, 1).reshape(3, NCAM * 3)),
            "wqT": np.ascontiguousarray(wq.T).astype(BF),
            "bq_r": bq.reshape(1, INNER).astype(BF),
            "w1T": np.ascontiguousarray(w_off1.T).astype(BF),
            "b1": b_off1.reshape(DIM, 1),
            "w2T": np.ascontiguousarray(w2p.T).astype(BF),
            "b2": b2p.reshape(1, 2 * NP).astype(BF),
            "wpT": np.ascontiguousarray(w_proj.T),
            "bp_r": b_proj.reshape(1, DIM).astype(BF),
            "cst01": np.concatenate([np.zeros((1, QPC), np.float32),
                                     np.ones((1, QPC), np.float32)], 0),
            "rep_in": (np.arange(128)[None, :] % 16 ==
                       np.arange(16)[:, None]).astype(np.float32),
        }
        in_maps.append(m)
    return in_maps


def assemble(results):
    """results: list of 8 dicts with out_l [QPC, DIM] -> [B, DIM, H, W]."""
    full = np.zeros((B, HW, DIM), dtype=np.float32)
    for core, r in enumerate(results):
        bc = core // (NCORES // B)
        q0 = (core % (NCORES // B)) * QPC
        full[bc, q0:q0 + QPC, :] = r["out_l"]
    return np.ascontiguousarray(full.transpose(0, 2, 1).reshape(B, DIM, H, W))


def kernel(**inputs):
    from concourse.bass_utils import run_bass_kernel_spmd
    nc = _build()
    in_maps = make_in_maps(inputs)
    res = run_bass_kernel_spmd(nc, in_maps, core_ids=list(range(NCORES)))
    return assemble(res.results)


if __name__ == "__main__":
    import reference
    inputs = {k: np.asarray(v) for k, v in reference.setup_inputs().items()}
    out = kernel(**inputs)
    exp = np.asarray(reference.reference(**{k: np.asarray(v) for k, v in inputs.items()}))
    err = np.abs(out - exp).max() / (np.abs(exp).max() + 1e-9)
    print("Relative error:", err)



# revision 42
# speedup vs baseline: 1.2264x; 1.2264x over previous
"""Trainium2 Bass kernel for CrossViewDeformableBlock (sparse deformable attention).

Contract: kernel(**inputs) -> np.ndarray takes FULL inputs (as from
setup_inputs()) and returns the FULL output [b, 128, 64, 64].

Sharding: 8 cores, q-parallel. Core c handles batch b_c = c//4 and query
range [(c%4)*1024, +1024) of the 64*64=4096 BEV queries. Each core builds
bf16 K|V image tables for its 6 cameras on-device, computes projection /
offsets / bilinear sample coordinates on-device, gathers paired 2-position
rows with dma_gather, bilinear-blends corners (split across the DVE and
Act engines), does the point-softmax attention and output projection, and
writes its [1024, 128] output shard. The host only slices inputs,
transposes weights, and casts the image features / kv weights to bf16
(layout/dtype staging).

Algebraic simplifications vs the reference:
  - the K bias (bkv[:128]) shifts sim by a constant per (q, head) and
    cancels in the softmax -> dropped entirely;
  - the V bias (bkv[128:]) is a constant output offset (attention weights
    sum to 1) -> folded into the output-projection bias via one matmul;
  - pinhole projection composed as a single 3x4 matrix MT = (K E[:3])^T
    on PE, applied per q-tile with one matmul for all 6 cams.

Emission order (chosen so engine queues never head-of-line block):
  1. offsets + projection matmuls, batched per-cam DVE coordinate math;
  2. kv tables for cams 0/1: one whole-image DMA, 22 matmuls into an SBUF
     stage, 3 large pattern DMAs out (paired-row interleave in the dram
     access pattern); tables 2-5 are built the same way, spread across
     the attention iterations two cameras ahead of their use;
  3. gather index tables: partition rewrap via 8 SBUF->SBUF DMAs plus a
     replication matmul reading a permuted access pattern;
  4. attention loop, cam-outer / q-tile-inner: dma_gather of 2KB paired
     corner rows -> x/y lerp blends (LERP2 custom DVE op for some points,
     Act-scale pairs + DVE add for others) -> K.q dot + point softmax
     (exp on Act, 1/NCAM folded into the Act-side att expansion that also
     makes the V-weight multiply a fast-mode stride-1 op) -> V sum;
  5. per-q-tile output projection emitted as soon as its last cam is done.
"""

import math
import os
import numpy as np

import concourse.bass as bass
import concourse.mybir as mybir
import concourse.tile as tile
from concourse import bacc
from concourse.bass import ts
from concourse.masks import make_identity

# ---------------------------------------------------------------- constants
B, NCAM, H, W = 2, 6, 64, 64
HW = H * W                      # 4096 queries per batch
IH, IW = 32, 88                 # image feature h, w
IHW = IH * IW                   # 2816 positions
HEADS, DH, INNER = 4, 32, 128
NP = 8                          # sample points per query
DIM = 128
NCORES = 8
QPC = HW // (NCORES // B)       # 1024 queries per core
NQT = QPC // 128                # 8 q-tiles of 128
PADROWS = 2944                  # 23 * 128 rows in kv table (2816 + 128 pad)
KVROW = 2 * INNER               # 256 channels (K|V) per position
F32 = mybir.dt.float32
BF16 = mybir.dt.bfloat16
I16 = mybir.dt.int16


def _build_lerp2_2x_uop():
    """Hand-written 2x_1p uop program for LERP2 (T1 per the custom-DVE design
    doc): lo = Src0*C0 + Src1*C1 on blocks 0-2, hi = Src0_HI*C0 + Src1_HI*C1
    on blocks 3-5, lo carried to the output on delay chain 0."""
    from concourse.dve_uop import (
        UopConfig, InpSel, OutSel, OutPath, AluOp, AluInp, DelayInp, Trigger,
        ENABLE,
    )
    u = UopConfig()
    u.enable_input(InpSel.SRC_0, 1)      # chain 0
    u.enable_input(InpSel.CONST_0, 2)    # chain 1
    u.enable_input(InpSel.SRC_1, 3)      # chain 2
    u.enable_input(InpSel.CONST_1, 4)    # chain 3
    u.enable_input(InpSel.SRC_0_HI, 5)   # chain 4
    u.enable_input(InpSel.SRC_1_HI, 6)   # chain 5
    u.require_inp0 = ENABLE
    u.require_inp1 = ENABLE
    u.trigger = (Trigger.SRC_TENSOR_DONE, Trigger.NONE, Trigger.NONE)
    b = u.datapath_config
    b[0].enable_alu(AluOp.MULTIPLY, AluInp.PREV_DELAY_0, AluInp.PREV_DELAY_1)
    b[0].pass_through_delay(1, 2, 3, 4, 5)
    b[1].enable_alu(AluOp.MULTIPLY, AluInp.PREV_DELAY_2, AluInp.PREV_DELAY_3)
    b[1].enable_delay_from_src(DelayInp.PREV_ALU_OUT, 0)
    b[1].pass_through_delay(1, 3, 4, 5)
    b[2].enable_alu(AluOp.ADD, AluInp.PREV_DELAY_0, AluInp.PREV_ALU_OUT)
    b[2].pass_through_delay(1, 3, 4, 5)
    b[3].enable_alu(AluOp.MULTIPLY, AluInp.PREV_DELAY_4, AluInp.PREV_DELAY_1)
    b[3].enable_delay_from_src(DelayInp.PREV_ALU_OUT, 0)
    b[3].pass_through_delay(3, 5)
    b[4].enable_alu(AluOp.MULTIPLY, AluInp.PREV_DELAY_5, AluInp.PREV_DELAY_3)
    b[4].enable_delay_from_src(DelayInp.PREV_ALU_OUT, 1)
    b[4].pass_through_delay(0)
    b[5].enable_alu(AluOp.ADD, AluInp.PREV_DELAY_1, AluInp.PREV_ALU_OUT)
    b[5].pass_through_delay(0)
    b[6].pass_through_alu()
    b[6].pass_through_delay(0)
    b[7].pass_through_alu()
    b[7].pass_through_delay(0)
    u.enable_output(OutSel.DELAY_0, OutPath.WR0_LO)
    u.enable_output(OutSel.ALU_OUT, OutPath.WR0_HI)
    return u


def _register_lerp_op():
    """Register LERP2 (out = in0*s0 + in1*s1, per-partition scalar columns)
    with both the stock 1x program and the hand-written 2x program."""
    from concourse.dve_spec import Spec, Src0, Src1, C0, C1, lower
    from concourse.dve_spec import _has_src1 as has_src1
    from concourse.dve_uop import DveOpSpec
    from concourse.dve_ops import (DveOp, OPS, _SUB_OPCODE_FOR_NAME,
                                   _CUSTOM_DVE_ROW_BASE, _COMPILE_CACHE,
                                   get_dve_sub_opcode)
    from concourse import dve_ops as _do

    spec = Spec(
        body=Src0 * C0 + Src1 * C1,
        reference=lambda in0, in1, s0, s1, imm2: (
            in0.astype(np.float32) * s0 + in1.astype(np.float32) * s1
        ),
    )
    if "LERP2" not in _SUB_OPCODE_FOR_NAME:
        opcode = _CUSTOM_DVE_ROW_BASE + len(OPS)
        assert opcode < 0x20
        OPS.append(DveOp("LERP2", spec, subdim=False, uops_sha={}, perf_en={}))
        _SUB_OPCODE_FOR_NAME["LERP2"] = opcode
        _do.CUSTOM_DVE_SPECS["LERP2"] = spec
    op = next(o for o in OPS if o.name == "LERP2")
    key = ("LERP2", "v3")  # TRN2 = v3
    if key not in _COMPILE_CACHE:
        compiled = DveOpSpec(
            name="LERP2",
            opcode=get_dve_sub_opcode("LERP2"),
            uops=lower(spec, ver="v3"),
            uops_2x=[_build_lerp2_2x_uop()],
            perf_max=1,
            rd1_en=has_src1(spec),
        )
        compiled.validate("v3")
        _COMPILE_CACHE[key] = compiled
    return op


def build_kernel(nc):
    """Emit the SPMD program. All per-core variation comes via input data."""
    lerp_op = _register_lerp_op()
    dotp_op = _register_dotp_op()

    # ---------------- dram I/O ----------------
    img = nc.dram_tensor("img", [NCAM, DIM, IHW], BF16, kind="ExternalInput").ap()
    wkvT = nc.dram_tensor("wkvT", [DIM, KVROW], BF16, kind="ExternalInput").ap()
    bv_c = nc.dram_tensor("bv_c", [INNER, 1], F32, kind="ExternalInput").ap()
    bev_l = nc.dram_tensor("bev_l", [DIM, QPC], BF16, kind="ExternalInput").ap()
    wxy_l = nc.dram_tensor("wxy_l", [2, QPC], F32, kind="ExternalInput").ap()
    E_l = nc.dram_tensor("E_l", [4, NCAM * 4], F32, kind="ExternalInput").ap()
    KT = nc.dram_tensor("KT", [3, NCAM * 3], F32, kind="ExternalInput").ap()
    wqT = nc.dram_tensor("wqT", [DIM, INNER], BF16, kind="ExternalInput").ap()
    bq_r = nc.dram_tensor("bq_r", [1, INNER], BF16, kind="ExternalInput").ap()
    w1T = nc.dram_tensor("w1T", [DIM, DIM], BF16, kind="ExternalInput").ap()
    b1 = nc.dram_tensor("b1", [DIM, 1], F32, kind="ExternalInput").ap()
    w2T = nc.dram_tensor("w2T", [DIM, 2 * NP], BF16, kind="ExternalInput").ap()
    b2 = nc.dram_tensor("b2", [1, 2 * NP], BF16, kind="ExternalInput").ap()
    wpT = nc.dram_tensor("wpT", [INNER, DIM], F32, kind="ExternalInput").ap()
    bp_r = nc.dram_tensor("bp_r", [1, DIM], BF16, kind="ExternalInput").ap()
    cst01 = nc.dram_tensor("cst01", [2, QPC], F32, kind="ExternalInput").ap()
    rep_in = nc.dram_tensor("rep_in", [16, 128], F32, kind="ExternalInput").ap()
    out_l = nc.dram_tensor("out_l", [QPC, DIM], F32, kind="ExternalOutput").ap()

    with tile.TileContext(nc) as tc:
        _emit(tc, nc, lerp_op, dotp_op, img, wkvT, bv_c, bev_l, wxy_l, E_l,
              KT, wqT, bq_r, w1T, b1, w2T, b2, wpT, bp_r, cst01, rep_in, out_l)
    return nc


def _lerp(nc, lerp_op, out, in0, in1, s0, s1):
    """out = in0*s0 + in1*s1 with s0/s1 [P,1] columns (2x perf mode)."""
    r = nc.vector._custom_dve(lerp_op, out=out, in0=in0, in1=in1, s0=s0, s1=s1)
    r.ins.perf_max = 1
    return r


def _build_dotp_uop(kind, two_x):
    """One DOTP uop: grouped dot product via segment-reset inclusive scan.
    kind: 'init' (acc=0, no consume) | 'steady' (acc+=prod) | 'reseed'
    (acc=prod at each subdim boundary). The out stream carries the running
    prefix every element; the caller's out AP picks where prefixes land."""
    from concourse.dve_uop import (
        UopConfig, InpSel, OutSel, OutPath, AluOp, AluInp, DelayInp,
        Trigger, ENABLE,
    )
    u = UopConfig()
    if two_x:
        u.enable_input(InpSel.SRC_0, 1)
        u.enable_input(InpSel.SRC_1, 2)
        u.enable_input(InpSel.SRC_0_HI, 3)
        u.enable_input(InpSel.SRC_1_HI, 4)
        u.enable_input(InpSel.ZERO, 5)
        zchain = AluInp.PREV_DELAY_4
        acc_blk = 3
    else:
        u.enable_input(InpSel.SRC_0, 1)
        u.enable_input(InpSel.SRC_1, 2)
        u.enable_input(InpSel.ZERO, 3)
        zchain = AluInp.PREV_DELAY_2
        acc_blk = 1
    if kind == "init":
        u.repeat_count = 1
        u.trigger = (Trigger.COUNT, Trigger.NONE, Trigger.NONE)
        u.next_uop = (1, 0, 0)
    else:
        u.require_inp0 = ENABLE
        u.require_inp1 = ENABLE
        u.enable_output(OutSel.ALU_OUT, OutPath.WR0_LO)
        if two_x:
            u.enable_output(OutSel.ALU_OUT, OutPath.WR0_HI)
        if kind == "steady":
            u.trigger = (Trigger.SRC_TENSOR_DONE, Trigger.SUB_DIM_DONE,
                         Trigger.NONE)
            u.next_uop = (0, 2, 0)
        else:  # reseed
            u.repeat_count = 1
            u.trigger = (Trigger.SRC_TENSOR_DONE, Trigger.SUB_DIM_DONE,
                         Trigger.COUNT)
            u.next_uop = (0, 2, 1)
    b = u.datapath_config
    b[0].enable_alu(AluOp.MULTIPLY, AluInp.PREV_DELAY_0, AluInp.PREV_DELAY_1)
    if two_x:
        b[0].pass_through_delay(2, 3, 4)
        b[1].enable_alu(AluOp.MULTIPLY, AluInp.PREV_DELAY_2, AluInp.PREV_DELAY_3)
        b[1].enable_delay_from_src(DelayInp.PREV_ALU_OUT, 0)
        b[1].pass_through_delay(4)
        b[2].enable_alu(AluOp.ADD, AluInp.PREV_DELAY_0, AluInp.PREV_ALU_OUT)
        b[2].pass_through_delay(4)
    else:
        b[0].pass_through_delay(2)
    if kind == "init":
        b[acc_blk].enable_alu(AluOp.BYPASS, zchain, zchain)
    elif kind == "steady":
        b[acc_blk].enable_alu(AluOp.ADD, AluInp.CURR_ALU_OUT,
                              AluInp.PREV_ALU_OUT)
    else:
        b[acc_blk].enable_alu(AluOp.BYPASS, AluInp.PREV_ALU_OUT,
                              AluInp.PREV_ALU_OUT)
    for i in range(acc_blk + 1, 8):
        b[i].pass_through_alu()
    return u


def _register_dotp_op():
    """Register DOTP (grouped dot product, 1x + 2x programs)."""
    from concourse.dve_spec import Spec, Src0, Src1, scan
    from concourse.dve_uop import DveOpSpec, AluOp
    from concourse.dve_ops import (DveOp, OPS, _SUB_OPCODE_FOR_NAME,
                                   _CUSTOM_DVE_ROW_BASE, _COMPILE_CACHE,
                                   get_dve_sub_opcode)
    from concourse import dve_ops as _do

    spec = Spec(
        body=scan(AluOp.ADD, Src0 * Src1),
        reference=lambda in0, in1, s0, s1, imm2: np.cumsum(
            in0.astype(np.float32) * in1.astype(np.float32), axis=-1
        ),
    )
    if "DOTP" not in _SUB_OPCODE_FOR_NAME:
        opcode = _CUSTOM_DVE_ROW_BASE + len(OPS)
        assert opcode < 0x20
        OPS.append(DveOp("DOTP", spec, subdim=True, uops_sha={}, perf_en={}))
        _SUB_OPCODE_FOR_NAME["DOTP"] = opcode
        _do.CUSTOM_DVE_SPECS["DOTP"] = spec
    op = next(o for o in OPS if o.name == "DOTP")
    key = ("DOTP", "v3")
    if key not in _COMPILE_CACHE:
        compiled = DveOpSpec(
            name="DOTP",
            opcode=get_dve_sub_opcode("DOTP"),
            uops=[_build_dotp_uop("init", False),
                  _build_dotp_uop("steady", False),
                  _build_dotp_uop("reseed", False)],
            uops_2x=[_build_dotp_uop("init", True),
                     _build_dotp_uop("steady", True),
                     _build_dotp_uop("reseed", True)],
            perf_max=1,
            rd1_en=True,
        )
        compiled.validate("v3")
        _COMPILE_CACHE[key] = compiled
    return op


def _dotp(nc, op, out, in0, in1):
    r = nc.vector._custom_dve(op, out=out, in0=in0, in1=in1)
    r.ins.perf_max = 1
    return r


def _emit(tc, nc, lerp_op, dotp_op, img, wkvT, bv_c, bev_l, wxy_l, E_l, KT,
          wqT, bq_r, w1T, b1, w2T, b2, wpT, bp_r, cst01, rep_in, out_l):
    import contextlib
    ctx = contextlib.ExitStack()
    with ctx:
        singles = ctx.enter_context(tc.tile_pool(name="singles", bufs=1))
        cpool = ctx.enter_context(tc.tile_pool(name="cpool", bufs=1))
        temps = ctx.enter_context(tc.tile_pool(name="temps", bufs=2))
        stpool = ctx.enter_context(tc.tile_pool(name="stpool", bufs=1))
        gath = ctx.enter_context(tc.tile_pool(name="gath", bufs=4))
        blend = ctx.enter_context(tc.tile_pool(name="blend", bufs=2))
        stats = ctx.enter_context(tc.tile_pool(name="stats", bufs=4))
        psum = ctx.enter_context(tc.tile_pool(name="psum", bufs=3, space="PSUM"))
        psum2 = ctx.enter_context(tc.tile_pool(name="psum2", bufs=2, space="PSUM"))
        dram = ctx.enter_context(tc.tile_pool(name="dram", bufs=1, space="DRAM"))

        AX = mybir.AxisListType
        ALU = mybir.AluOpType
        ACTF = mybir.ActivationFunctionType

        # ------------- resident tiles -------------
        ident = singles.tile([128, 128], F32)
        make_identity(nc, ident[:])
        wkvT_sb = singles.tile([DIM, KVROW], BF16)
        nc.sync.dma_start(out=wkvT_sb[:], in_=wkvT)
        bvc_sb = singles.tile([INNER, 1], F32)
        nc.sync.dma_start(out=bvc_sb[:], in_=bv_c)
        bev_sb = singles.tile([DIM, QPC], BF16)
        nc.sync.dma_start(out=bev_sb[:], in_=bev_l)
        wqT_sb = singles.tile([DIM, INNER], BF16)
        nc.sync.dma_start(out=wqT_sb[:], in_=wqT)
        bq_sb = singles.tile([1, INNER], BF16)
        nc.sync.dma_start(out=bq_sb[:], in_=bq_r)
        w1T_sb = singles.tile([DIM, DIM], BF16)
        nc.sync.dma_start(out=w1T_sb[:], in_=w1T)
        w2T_sb = singles.tile([DIM, 2 * NP], BF16)
        nc.sync.dma_start(out=w2T_sb[:], in_=w2T)
        wpT_sb = singles.tile([INNER, DIM], F32)
        nc.sync.dma_start(out=wpT_sb[:], in_=wpT)
        bp_sb = singles.tile([1, DIM], BF16)
        nc.sync.dma_start(out=bp_sb[:], in_=bp_r)
        b1_sb = singles.tile([DIM, 1], F32)
        nc.sync.dma_start(out=b1_sb[:], in_=b1)
        b2r_sb = singles.tile([1, 2 * NP], BF16)
        nc.sync.dma_start(out=b2r_sb[:], in_=b2)
        E_sb = singles.tile([4, NCAM * 4], F32)
        nc.sync.dma_start(out=E_sb[:], in_=E_l)
        KT_sb = singles.tile([3, NCAM * 3], F32)
        nc.sync.dma_start(out=KT_sb[:], in_=KT)
        REP_sb = singles.tile([16, 128], F32)
        nc.sync.dma_start(out=REP_sb[:], in_=rep_in)
        ones_bf = singles.tile([1, 128], BF16)
        nc.vector.memset(ones_bf[:], 1.0)

        # xyz1 = [wx, wy, 0, 1]
        xyz1_sb = singles.tile([4, QPC], F32)
        nc.sync.dma_start(out=xyz1_sb[:2, :], in_=wxy_l)
        nc.sync.dma_start(out=xyz1_sb[2:4, :], in_=cst01)

        # resident products of phase A
        qbf_sb = singles.tile([128, QPC], BF16)
        off_t_all = singles.tile([128, NQT * 16], F32)
        wA_sb = singles.tile([128, NCAM * NQT * 16], F32)
        wB_sb = singles.tile([128, NCAM * NQT * 16], F32)
        idx2_all = singles.tile([128, NCAM * NQT * NP], F32)
        wacc_all = singles.tile([128, NQT * INNER], F32)
        T_tiles = [singles.tile([128, NCAM * 64], I16, tag=f"Tq{qt}",
                                name=f"Tq{qt}")
                   for qt in range(NQT)]

        # per-cam kv tables in DRAM; row y*IW+x holds KV(y,x) ++ KV(y+1,x)
        kv_cam = [dram.tile([PADROWS, 2 * KVROW], BF16, tag=f"kv{c}",
                            name=f"kv{c}")
                  for c in range(NCAM)]

        # ---------------- kv table builder ----------------
        zt = singles.tile([128, 2 * KVROW], BF16)
        nc.vector.memset(zt[:], 0)
        NPT = IHW // 128  # 22 position tiles

        # zero the tail pad rows (IHW..PADROWS) of every cam upfront
        for _c in range(NCAM):
            kd = kv_cam[_c]
            nc.sync.dma_start(out=kd[IHW:PADROWS, :], in_=zt[:])

        # Table build, partition-blocked: partition p of the SBUF stage holds
        # table rows p*NPT..p*NPT+NPT-1 (col t = row p*NPT+t), so the store
        # is one DMA of 128 fully contiguous ~22KB runs. The paired second
        # half (row r ++ row r+IW) comes from a second matmul over an
        # IW-shifted column slice of the image. Store goes through the
        # gpsimd software DGE so it spreads across all 16 DMA rings instead
        # of serializing the sync/act sequencer queues.
        build_state = {}
        ROWE = 2 * KVROW  # 512 elements per table row

        def build_start(cam):
            # img padded with a zero tail so the +IW shifted slice stays
            # in-bounds (positions IHW..IHW+128 read as zero -> KV pad = 0)
            img_sb = stpool.tile([128, IHW + 128], BF16, tag="imgsb",
                                 name="imgsb")
            nc.vector.memset(img_sb[:, IHW:], 0)
            nc.scalar.dma_start(out=img_sb[:, 0:IHW], in_=img[cam])
            stage = stpool.tile([128, NPT, ROWE], BF16, tag="stage",
                                name="stage")
            build_state[cam] = (img_sb, stage)

        def build_chunk(cam, lo, hi, split_copies=False):
            img_sb, stage = build_state[cam]
            iap = img_sb[:]
            for t in range(lo, min(hi, NPT)):
                kv_ps = psum.tile([128, ROWE], F32, tag="mm2")
                # positions p*NPT+t (stride-NPT column slice), and +IW pair
                for j, off in enumerate((t, t + IW)):
                    lhsT = bass.AP(tensor=img_sb.tensor,
                                   offset=iap.offset + off,
                                   ap=[iap.ap[0], [NPT, 128]])
                    nc.tensor.matmul(out=kv_ps[:, ts(j, KVROW)], lhsT=lhsT,
                                     rhs=wkvT_sb[:], start=True, stop=True)
                if split_copies and t % 2 == 0:
                    nc.vector.tensor_copy(out=stage[:, t, :], in_=kv_ps[:])
                else:
                    nc.scalar.activation(out=stage[:, t, :], in_=kv_ps[:],
                                         func=ACTF.Copy)

        def build_store(cam):
            img_sb, stage = build_state.pop(cam)
            kd = kv_cam[cam]
            sap = stage[:]
            # dram row p*NPT+t at offset (p*NPT+t)*ROWE: per partition one
            # contiguous NPT*ROWE run
            out1 = bass.AP(tensor=kd.tensor, offset=kd[:].offset,
                           ap=[[NPT * ROWE, 128], [1, NPT * ROWE]])
            in1 = bass.AP(tensor=stage.tensor, offset=sap.offset,
                          ap=[sap.ap[0], [1, NPT * ROWE]])
            nc.gpsimd.dma_start(out=out1, in_=in1)

        # cam 0/1 tables first: their matmuls/copies/DMAs have no deps on
        # phase A, so PE/Act/DMA stream them at full rate while nothing else
        # is ready; phase A's engine ping-pong then overlaps the store DMAs
        # offsets: o1 = relu(w1 @ bev + b1); off = w2 @ o1 + b2  [16, QPC]
        o1_sb = singles.tile([DIM, QPC], BF16)
        for hf in range(2):
            o1_ps = psum2.tile([DIM, QPC // 2], F32, tag="wide")
            nc.tensor.matmul(out=o1_ps[:], lhsT=w1T_sb[:],
                             rhs=bev_sb[:, ts(hf, QPC // 2)], start=True, stop=True)
            nc.scalar.activation(out=o1_sb[:, ts(hf, QPC // 2)], in_=o1_ps[:],
                                 func=ACTF.Relu, bias=b1_sb[:], scale=1.0)
        # transposed offsets directly: off_t[q, o] = sum_d o1[d,q] w2T[d,o]
        # + b2 (bias accumulated via the ones-row matmul trick)
        for qt in range(NQT):
            ot_ps = psum.tile([128, 2 * NP], F32, tag="mm")
            nc.tensor.matmul(out=ot_ps[:], lhsT=ones_bf[:], rhs=b2r_sb[:],
                             start=True, stop=False)
            nc.tensor.matmul(out=ot_ps[:], lhsT=o1_sb[:, ts(qt, 128)],
                             rhs=w2T_sb[:], start=False, stop=True)
            nc.vector.tensor_copy(out=off_t_all[:, ts(qt, 2 * NP)],
                                  in_=ot_ps[:])

        # ---------------- A: per-cam projection + coords ----------------
        BIGF = 8388608.0
        halfx = 0.5 * (IW - 1)
        halfy = 0.5 * (IH - 1)
        # MT = (K @ E[:3,:])^T [4,3] per cam, computed directly:
        # MT[j,i] = sum_k E[k,j] K[i,k] = matmul(lhsT=E_rows, rhs=K^T)
        MT_sb = singles.tile([4, NCAM * 3], F32)
        for cam in range(NCAM):
            mt_ps = psum.tile([4, 3], F32, tag="mm")
            nc.tensor.matmul(out=mt_ps[:], lhsT=E_sb[0:3, ts(cam, 4)],
                             rhs=KT_sb[:, ts(cam, 3)], start=True, stop=True)
            nc.scalar.activation(out=MT_sb[:, ts(cam, 3)], in_=mt_ps[:],
                                 func=ACTF.Copy)

        # pxt_all [128, (qt, cam, 3)]: all cams' projections in one matmul/qt
        pxt_all = singles.tile([128, NQT * NCAM * 3], F32)
        for qt in range(NQT):
            pt_ps = psum.tile([128, NCAM * 3], F32, tag="mm")
            nc.tensor.matmul(out=pt_ps[:], lhsT=xyz1_sb[:, ts(qt, 128)],
                             rhs=MT_sb[:], start=True, stop=True)
            nc.vector.tensor_copy(out=pxt_all[:, ts(qt, NCAM * 3)],
                                  in_=pt_ps[:])

        pap = pxt_all[:]
        oap = off_t_all[:]

        def emit_coords():
            # all NCAM cameras in one batched DVE chain (minimizes the
            # dependent-op latency on the startup critical path)
            NC6 = NCAM

            def _px(col):  # [128, (cam, qt)] slice of pxt_all (qt, cam, 3)
                return bass.AP(tensor=pxt_all.tensor,
                               offset=pap.offset + col,
                               ap=[pap.ap[0], [3, NC6], [NCAM * 3, NQT]])

            # rden = 1 / max(pz, 1e-6)
            rden = cpool.tile([128, NC6, NQT], F32, tag="rden")
            nc.vector.tensor_scalar(out=rden[:], in0=_px(2), scalar1=1e-6,
                                    scalar2=None, op0=ALU.max)
            nc.vector.reciprocal(out=rden[:], in_=rden[:])
            # g = uv/(dim-1)*2 - 1
            gx = cpool.tile([128, NC6, NQT], F32, tag="gx")
            nc.vector.tensor_tensor(out=gx[:], in0=_px(0), in1=rden[:],
                                    op=ALU.mult)
            nc.vector.tensor_scalar(out=gx[:], in0=gx[:],
                                    scalar1=2.0 / (IW - 1), scalar2=1.0,
                                    op0=ALU.mult, op1=ALU.subtract)
            gy = cpool.tile([128, NC6, NQT], F32, tag="gy")
            nc.vector.tensor_tensor(out=gy[:], in0=_px(1), in1=rden[:],
                                    op=ALU.mult)
            nc.vector.tensor_scalar(out=gy[:], in0=gy[:],
                                    scalar1=2.0 / (IH - 1), scalar2=1.0,
                                    op0=ALU.mult, op1=ALU.subtract)

            # sxy [128, (cam, qt, 16)]: samp = clip(off+g, -1, 1) -> pixels
            sxy = cpool.tile([128, NC6 * NQT * 16], F32, tag="sxy")
            sap = sxy[:]

            def _sl(t, tap, off0):  # [128, (cam, qt, 8)] x(0)/y(8) slices
                return bass.AP(tensor=t.tensor, offset=tap.offset + off0,
                               ap=[tap.ap[0], [NQT * 16, NC6], [16, NQT],
                                   [1, NP]])

            def _obc(off0):  # off_t_all bc over the cams
                return bass.AP(tensor=off_t_all.tensor,
                               offset=oap.offset + off0,
                               ap=[oap.ap[0], [0, NC6], [16, NQT], [1, NP]])

            def _gbc(g):  # gx/gy bc over p
                gp = g[:]
                return bass.AP(tensor=g.tensor, offset=gp.offset,
                               ap=[gp.ap[0], [NQT, NC6], [1, NQT], [0, NP]])

            nc.vector.tensor_tensor(out=_sl(sxy, sap, 0), in0=_obc(0),
                                    in1=_gbc(gx), op=ALU.add)
            nc.vector.tensor_tensor(out=_sl(sxy, sap, NP), in0=_obc(NP),
                                    in1=_gbc(gy), op=ALU.add)
            nc.vector.tensor_scalar(out=sxy[:], in0=sxy[:], scalar1=1.0,
                                    scalar2=-1.0, op0=ALU.min, op1=ALU.max)
            nc.vector.tensor_scalar(out=_sl(sxy, sap, 0), in0=_sl(sxy, sap, 0),
                                    scalar1=1.0, scalar2=halfx,
                                    op0=ALU.add, op1=ALU.mult)
            nc.vector.tensor_scalar(out=_sl(sxy, sap, NP), in0=_sl(sxy, sap, NP),
                                    scalar1=1.0, scalar2=halfy,
                                    op0=ALU.add, op1=ALU.mult)

            # floor via +2^23 round-to-nearest, then fixup so frac >= 0
            rnd = cpool.tile([128, NC6 * NQT * 16], F32, tag="rnd")
            nc.vector.tensor_scalar(out=rnd[:], in0=sxy[:], scalar1=BIGF,
                                    scalar2=BIGF, op0=ALU.add, op1=ALU.subtract)
            dfr = cpool.tile([128, NC6 * NQT * 16], F32, tag="dfr")
            nc.vector.tensor_tensor(out=dfr[:], in0=sxy[:], in1=rnd[:],
                                    op=ALU.subtract)
            msk = cpool.tile([128, NC6 * NQT * 16], F32, tag="msk")
            nc.vector.tensor_scalar(out=msk[:], in0=dfr[:], scalar1=0.0,
                                    scalar2=None, op0=ALU.is_lt)
            x0y0 = sxy  # sxy is dead after dfr; reuse its buffer
            nc.vector.tensor_tensor(out=x0y0[:], in0=rnd[:], in1=msk[:],
                                    op=ALU.subtract)
            nc.vector.tensor_tensor(out=wB_sb[:, 0:NC6 * NQT * 16],
                                    in0=dfr[:], in1=msk[:], op=ALU.add)
            nc.vector.tensor_scalar(out=wA_sb[:, 0:NC6 * NQT * 16],
                                    in0=wB_sb[:, 0:NC6 * NQT * 16],
                                    scalar1=-1.0, scalar2=1.0,
                                    op0=ALU.mult, op1=ALU.add)
            # idx = y0*IW + x0 (local per cam); idx2_all layout (qt, cam, p)
            xap = x0y0[:]
            rap = rnd[:]  # rnd is dead after x0y0; reuse its buffer for tmp
            tmp = bass.AP(tensor=rnd.tensor, offset=rap.offset,
                          ap=[rap.ap[0], [NQT * NP, NC6], [NP, NQT], [1, NP]])
            nc.vector.tensor_scalar(out=tmp, in0=_sl(x0y0, xap, NP),
                                    scalar1=float(IW), scalar2=None,
                                    op0=ALU.mult)
            i2 = idx2_all[:]
            idst = bass.AP(tensor=idx2_all.tensor, offset=i2.offset,
                           ap=[i2.ap[0], [NP, NC6], [NCAM * NP, NQT], [1, NP]])
            nc.vector.tensor_tensor(out=idst, in0=tmp,
                                    in1=_sl(x0y0, xap, 0), op=ALU.add)

        # ---------------- A: queries ----------------
        for qt in range(NQT):
            q_ps = psum.tile([128, INNER], F32, tag="mm")
            nc.tensor.matmul(out=q_ps[:], lhsT=ones_bf[:], rhs=bq_sb[:],
                             start=True, stop=False)
            nc.tensor.matmul(out=q_ps[:], lhsT=bev_sb[:, ts(qt, 128)],
                             rhs=wqT_sb[:], start=False, stop=True)
            nc.scalar.activation(out=qbf_sb[:, ts(qt, INNER)], in_=q_ps[:],
                                 func=ACTF.Copy)

        # fold the V bias through the output projection: bpp = bp + bv @ wpT
        # (bk cancels in the softmax; bv is a constant output offset since the
        # attention weights sum to 1)
        bvp_ps = psum.tile([1, DIM], F32, tag="mm")
        nc.tensor.matmul(out=bvp_ps[:], lhsT=bvc_sb[:], rhs=wpT_sb[:],
                         start=True, stop=True)
        bpp_sb = singles.tile([1, DIM], BF16)
        nc.vector.tensor_tensor(out=bpp_sb[:], in0=bvp_ps[:], in1=bp_sb[:],
                                op=ALU.add)

        # ---------------- B: gather index tables (per cam-pair) ----------
        # Need T[16k+pl, cam*64 + p*8 + qh] = idx2_all[qh*16+pl, (qt, cam, p)].
        # Per pair: 8 SBUF->SBUF DMAs rewrap partitions into a [16,
        # (qh,qt,cam2,p)] scratch; a replication matmul reading a permuted
        # access pattern writes the pair's T_tiles columns. Pair 0 runs
        # before the loop; pairs 1/2 are emitted inside the first camera's
        # iterations (their coords overlap the first gathers).
        tsc = singles.tile([16, 8 * NQT * NCAM * NP], F32)

        def emit_rewrap():
            for qh in range(8):
                nc.sync.dma_start(
                    out=tsc[:, qh * 384:(qh + 1) * 384],
                    in_=idx2_all[qh * 16:(qh + 1) * 16, :])

        def emit_ttables(qts):
            tap = tsc[:]
            for qt in qts:
                rhs_perm = bass.AP(tensor=tsc.tensor,
                                   offset=tap.offset + qt * 48,
                                   ap=[tap.ap[0], [NP, NCAM], [1, NP],
                                       [384, 8]])
                rep_ps = psum2.tile([128, NCAM * 64], F32, tag="wide")
                nc.tensor.matmul(out=rep_ps[:], lhsT=REP_sb[:],
                                 rhs=rhs_perm, start=True, stop=True)
                nc.vector.tensor_copy(out=T_tiles[qt][:], in_=rep_ps[:])

        # startup pipeline: batched coords for all cams overlap the cam-0
        # table build (coords on DVE, build on PE/Act/V-copies)
        emit_coords()
        build_start(0)
        build_chunk(0, 0, NPT)
        build_store(0)
        emit_rewrap()
        emit_ttables(list(range(NQT)))
        build_start(1)
        build_chunk(1, 0, NPT)
        build_store(1)

        # ---------------- C/D/E: attention, cam-outer ----------------
        # cam c+1's kv table tiles are emitted spread across cam c's q-tile
        # iterations so their psum->sbuf copies never head-of-line-block the
        # Act engine ahead of the softmax exp.
        # All blends run as 2x-mode LERP2 on the DVE; the Act engine only
        # handles exp/atx/table copies.

        for cam in range(NCAM):
            camv = kv_cam[cam]
            kv_view = bass.AP(tensor=camv.tensor, offset=camv[:].offset,
                              ap=[[2 * KVROW, PADROWS - 1], [1, 4 * KVROW]])
            for qt in range(NQT):
                if cam + 2 < NCAM:
                    # table c+2 spread over cam c's iterations: it is complete
                    # one full camera before its gathers start, so the gather
                    # prefetch never stalls on a table write
                    if qt == 0:
                        build_start(cam + 2)
                    build_chunk(cam + 2, qt * 3, qt * 3 + 3)
                    if qt == NQT - 1:
                        build_store(cam + 2)
                kvraw = gath.tile([128, NP, 4 * KVROW], BF16, tag="kvraw")
                nc.gpsimd.dma_gather(
                    out_ap=kvraw[:], in_ap=kv_view,
                    idxs_ap=T_tiles[qt][:, ts(cam, 64)],
                    num_idxs=1024, num_idxs_reg=1024,
                    elem_size=4 * KVROW, elem_step=2 * KVROW,
                    single_packet=False)
                # x-blend: 8 rows of 512 [(y0,y1) x (K|V)]
                blkw = (cam * NQT + qt) * 16
                kvx = blend.tile([128, NP, 2 * KVROW], BF16, tag="kvx")
                for p in range(NP):
                    sa = wA_sb[:, blkw + p:blkw + p + 1]
                    sb = wB_sb[:, blkw + p:blkw + p + 1]
                    _lerp(nc, lerp_op, kvx[:, p, :],
                          kvraw[:, p, 0:2 * KVROW],
                          kvraw[:, p, 2 * KVROW:4 * KVROW], sa, sb)
                # y-blend: 8 points of 256, written as split K/V planes
                # (kvb2[:,0] = K [128, NP, DH*HEADS] contiguous, kvb2[:,1] = V)
                kvb2 = blend.tile([128, 2, NP, INNER], BF16, tag="kvb2")
                k2ap = kvb2[:]
                for p in range(NP):
                    sa = wA_sb[:, blkw + 8 + p:blkw + 9 + p]
                    sb = wB_sb[:, blkw + 8 + p:blkw + 9 + p]
                    yout = bass.AP(tensor=kvb2.tensor,
                                   offset=k2ap.offset + p * INNER,
                                   ap=[k2ap.ap[0], [NP * INNER, 2], [1, INNER]])
                    if p >= 5:
                        tA = blend.tile([128, KVROW], BF16, tag="ya")
                        nc.scalar.activation(out=tA[:], in_=kvx[:, p, 0:KVROW],
                                             func=ACTF.Copy, scale=sa)
                        tB = blend.tile([128, KVROW], BF16, tag="yb")
                        nc.scalar.activation(out=tB[:],
                                             in_=kvx[:, p, KVROW:2 * KVROW],
                                             func=ACTF.Copy, scale=sb)
                        nc.vector.tensor_tensor(out=yout, in0=tA[:],
                                                in1=tB[:], op=ALU.add)
                    else:
                        _lerp(nc, lerp_op, yout,
                              kvx[:, p, 0:KVROW], kvx[:, p, KVROW:2 * KVROW],
                              sa, sb)
                # sim via fused 2x dot-product scan: running prefix lands in
                # simsc; group sums (p,h) at column g*DH+DH-1
                simsc = blend.tile([128, NP * INNER], BF16, tag="simsc")
                qv = qbf_sb[:, ts(qt, INNER)]
                _dotp(nc, dotp_op, simsc[:],
                      bass.AP(tensor=kvb2.tensor, offset=k2ap.offset,
                              ap=[k2ap.ap[0], [DH, NP * HEADS], [1, DH]]),
                      bass.AP(tensor=qbf_sb.tensor, offset=qv.offset,
                              ap=[qv.ap[0], [0, NP], [1, INNER]]))
                # softmax over p; per-head normalization folded into the
                # Act-engine expansion scale (1/NCAM is folded into wkv's V
                # half on the host)
                scap = simsc[:]
                esim = stats.tile([128, NP, HEADS], BF16, tag="esim")
                nc.scalar.activation(
                    out=esim[:],
                    in_=bass.AP(tensor=simsc.tensor, offset=scap.offset + DH - 1,
                                ap=[scap.ap[0], [DH, NP * HEADS]]),
                    func=ACTF.Exp)
                ssum = stats.tile([128, HEADS], F32, tag="ssum")
                esap = esim[:]
                nc.vector.tensor_reduce(
                    out=ssum[:],
                    in_=bass.AP(tensor=esim.tensor, offset=esap.offset,
                                ap=[esap.ap[0], [1, HEADS], [HEADS, NP]]),
                    axis=AX.X, op=ALU.add)
                srec = stats.tile([128, HEADS], F32, tag="srec")
                nc.vector.reciprocal(out=srec[:], in_=ssum[:])
                # expand att = esim/ssum over DH: one scaled copy per head
                atx = blend.tile([128, NP, HEADS, DH], BF16, tag="atx")
                atxap = atx[:]
                for h in range(HEADS):
                    nc.scalar.activation(
                        out=bass.AP(tensor=atx.tensor,
                                    offset=atxap.offset + h * DH,
                                    ap=[atxap.ap[0], [HEADS * DH, NP], [1, DH]]),
                        in_=bass.AP(tensor=esim.tensor, offset=esap.offset + h,
                                    ap=[esap.ap[0], [HEADS, NP], [0, DH]]),
                        func=ACTF.Copy, scale=srec[:, h:h + 1])
                vw = blend.tile([128, NP, INNER], BF16, tag="vw")
                nc.vector.tensor_tensor(out=vw[:], in0=kvb2[:, 1, :, :],
                                        in1=atx[:], op=ALU.mult)
                # tree-sum over the 8 points
                t1 = stats.tile([128, 4, INNER], BF16, tag="t1")
                nc.vector.tensor_tensor(out=t1[:], in0=vw[:, 0:4, :],
                                        in1=vw[:, 4:8, :], op=ALU.add)
                t2 = stats.tile([128, 2, INNER], BF16, tag="t2")
                nc.vector.tensor_tensor(out=t2[:], in0=t1[:, 0:2, :],
                                        in1=t1[:, 2:4, :], op=ALU.add)
                if cam == 0:
                    nc.vector.tensor_tensor(out=wacc_all[:, ts(qt, INNER)],
                                            in0=t2[:, 0, :], in1=t2[:, 1, :],
                                            op=ALU.add)
                else:
                    wsum = stats.tile([128, INNER], BF16, tag="wsum")
                    nc.vector.tensor_tensor(out=wsum[:], in0=t2[:, 0, :],
                                            in1=t2[:, 1, :], op=ALU.add)
                    nc.vector.tensor_tensor(out=wacc_all[:, ts(qt, INNER)],
                                            in0=wacc_all[:, ts(qt, INNER)],
                                            in1=wsum[:], op=ALU.add)
                if cam == NCAM - 1:
                    # output projection for this q-tile, overlapped with the
                    # remaining iterations
                    wt_ps = psum.tile([128, 128], F32, tag="mm")
                    nc.tensor.transpose(out=wt_ps[:],
                                        in_=wacc_all[:, ts(qt, INNER)],
                                        identity=ident[:])
                    waccT = temps.tile([128, 128], F32, tag="waccT")
                    nc.scalar.activation(out=waccT[:], in_=wt_ps[:],
                                         func=ACTF.Copy)
                    out_ps = psum.tile([128, DIM], F32, tag="mm")
                    nc.tensor.matmul(out=out_ps[:], lhsT=ones_bf[:],
                                     rhs=bpp_sb[:], start=True, stop=False)
                    nc.tensor.matmul(out=out_ps[:], lhsT=waccT[:],
                                     rhs=wpT_sb[:], start=False, stop=True)
                    outf = temps.tile([128, DIM], F32, tag="outf")
                    nc.scalar.activation(out=outf[:], in_=out_ps[:],
                                         func=ACTF.Copy)
                    nc.sync.dma_start(out=out_l[ts(qt, 128), :], in_=outf[:])


# ---------------------------------------------------------------- host side
_CACHED = {}


def _build():
    if "nc" not in _CACHED:
        nc = bacc.Bacc("TRN2", target_bir_lowering=False, debug=False,
                       num_devices=NCORES)
        build_kernel(nc)
        nc.compile()
        _CACHED["nc"] = nc
    return _CACHED["nc"]


def make_in_maps(inputs):
    """Slice/transpose/cast FULL inputs into 8 per-core input dicts."""
    import ml_dtypes
    BF = ml_dtypes.bfloat16
    f = lambda x: np.ascontiguousarray(np.asarray(x, dtype=np.float32))
    bev = f(inputs["bev"]).reshape(B, DIM, HW)
    img_feats = f(inputs["img_feats"]).reshape(B, NCAM, DIM, IHW)
    Kc = f(inputs["K"])
    Ec = f(inputs["E"])
    world_xy = f(inputs["world_xy"]).reshape(2, HW)
    wq = f(inputs["wq"]); bq = f(inputs["bq"])
    wkv = f(inputs["wkv"]); bkv = f(inputs["bkv"])
    w_off1 = f(inputs["w_off1"]); b_off1 = f(inputs["b_off1"])
    w_off2 = f(inputs["w_off2"]); b_off2 = f(inputs["b_off2"])
    w_proj = f(inputs["w_proj"]); b_proj = f(inputs["b_proj"])

    # row-permute w_off2/b_off2 from (p, c) to (c, p) ordering
    perm = [p * 2 + c for c in range(2) for p in range(NP)]
    w2p = w_off2[perm, :]
    b2p = b_off2[perm]

    # fold the 1/NCAM camera mean into the V projection (attention weights
    # sum to 1 per cam, so only V and its bias carry the mean; the bias fold
    # bv @ wpT is unaffected since sum över cams restores the factor NCAM)
    wkv_f = wkv.copy()
    wkv_f[INNER:, :] *= 1.0 / NCAM

    in_maps = []
    for core in range(NCORES):
        bc = core // (NCORES // B)
        q0 = (core % (NCORES // B)) * QPC
        m = {
            "img": np.ascontiguousarray(img_feats[bc]).astype(BF),
            "wkvT": np.ascontiguousarray(wkv_f.T).astype(BF),
            "bv_c": bkv[INNER:].reshape(INNER, 1),
            "bev_l": np.ascontiguousarray(bev[bc, :, q0:q0 + QPC]).astype(BF),
            "wxy_l": np.ascontiguousarray(world_xy[:, q0:q0 + QPC]),
            "E_l": np.ascontiguousarray(Ec[bc].transpose(1, 0, 2).reshape(4, NCAM * 4)),
            "KT": np.ascontiguousarray(Kc[bc].transpose(2, 0, 1).reshape(3, NCAM * 3)),
            "wqT": np.ascontiguousarray(wq.T).astype(BF),
            "bq_r": bq.reshape(1, INNER).astype(BF),
            "w1T": np.ascontiguousarray(w_off1.T).astype(BF),
            "b1": b_off1.reshape(DIM, 1),
            "w2T": np.ascontiguousarray(w2p.T).astype(BF),
            "b2": b2p.reshape(1, 2 * NP).astype(BF),
            "wpT": np.ascontiguousarray(w_proj.T),
            "bp_r": b_proj.reshape(1, DIM).astype(BF),
            "cst01": np.concatenate([np.zeros((1, QPC), np.float32),
                                     np.ones((1, QPC), np.float32)], 0),
            "rep_in": (np.arange(128)[None, :] % 16 ==
                       np.arange(16)[:, None]).astype(np.float32),
        }
        in_maps.append(m)
    return in_maps


def assemble(results):
    """results: list of 8 dicts with out_l [QPC, DIM] -> [B, DIM, H, W]."""
    full = np.zeros((B, HW, DIM), dtype=np.float32)
    for core, r in enumerate(results):
        bc = core // (NCORES // B)
        q0 = (core % (NCORES // B)) * QPC
        full[bc, q0:q0 + QPC, :] = r["out_l"]
    return np.ascontiguousarray(full.transpose(0, 2, 1).reshape(B, DIM, H, W))


def kernel(**inputs):
    from concourse.bass_utils import run_bass_kernel_spmd
    nc = _build()
    in_maps = make_in_maps(inputs)
    res = run_bass_kernel_spmd(nc, in_maps, core_ids=list(range(NCORES)))
    return assemble(res.results)


if __name__ == "__main__":
    import reference
    inputs = {k: np.asarray(v) for k, v in reference.setup_inputs().items()}
    out = kernel(**inputs)
    exp = np.asarray(reference.reference(**{k: np.asarray(v) for k, v in inputs.items()}))
    err = np.abs(out - exp).max() / (np.abs(exp).max() + 1e-9)
    print("Relative error:", err)

